# revision 72
# baseline (speedup 1.0000x reference)
"""Distributed Trainium2 (Bass/Tile) kernel for masked GAT-style attention.

Reference computation (H=4 heads, N=4096 nodes, D=128):
    scores = leaky_relu(x @ W^T + b, 0.2)            # [H, N, N]
    att    = where(mask, softmax(where(mask, scores, -inf)), 0)
    out    = att @ x                                  # [H, N, D]

Sharding: 8 cores = 4 heads x 2 row-blocks of 2048 nodes. Each core
computes out[h, r0:r0+2048] independently (no collectives).

Per-core layout ("transposed scores"): scores^T tiles [m=128 part, n free]
so the PV matmul uses the attention tile directly as the stationary
operand and the softmax row-sum comes for free from an appended
ones-column on x.

The 32 m-tiles per chunk are processed as 16 PAIRS (every SBUF-side EW
op covers 2048 elements, halving per-instruction overhead), split into
three pipelines chosen to balance ACT (1.2 GHz, 1x) and DVE (0.96 GHz,
STT=1x, TT=2x @16-bit):
- A (3 pairs): ACT Prelu from PSUM per tile -> one shared pair Exp on
  ACT -> fp16 mask multiply on DVE (2x TT). Exact exp.
- E (10 pairs): ACT Prelu(scale=K) gives K*leaky(s) in fp16; one DVE
  pair TT-add with a host-baked fp16 mask tensor {B, B-30K} produces
  Schraudolph exp bits in uint16 (negative results saturate to 0 =
  masked-out entries vanish); bitcast fp16 feeds the PV directly.
- C (3 pairs): all-DVE. STT t=0.2K*s+mask per tile (PSUM read), then a
  pair STT u=max(5t,t) and a 4x-mode tensor_scalar add-B -> uint16
  Schraudolph bits.
Schraudolph tiles (E+C) trade ~2-3% per-entry exp noise (which largely
cancels in the softmax) for removing the ACT exp pass; measured
end-to-end rel err 1.39e-2 vs the 2e-2 gate.

Each pair's PV matmuls (and the A pairs' mask multiply) are deferred
five pairs behind the QK/EW front (a software pipeline queue, one pop
per iteration), so the Tensor queue never sits directly behind a long
EW chain and jitter doesn't couple through the 2-deep s_ps PSUM ring.

Mask ships once as uint16 [N, ROWS]: fp16 1.0/0.0 rows for A tiles,
fp16 {B, B-30K} for E, fp16 {0, -44320} for C; bitcast on chip.
Mask/xa DMAs are batched and issued from the otherwise-idle GpSimd
sequencer (alternating with Sync); wt/xt stream in 512-col pieces
staggered through chunk 0 so early mask batches aren't starved and the
first QK matmul starts as soon as the engines come up.
"""

import sys

if "/opt/trn_rl_repo" not in sys.path:
    sys.path.insert(0, "/opt/trn_rl_repo")

import numpy as np
import ml_dtypes

import concourse.bass as bass
import concourse.tile as tile
from concourse import bacc, mybir
from concourse.bass_utils import run_bass_kernel_spmd

BF = mybir.dt.bfloat16
F16 = mybir.dt.float16
F32 = mybir.dt.float32
U16 = mybir.dt.uint16
BF_NP = ml_dtypes.bfloat16

H, N, D = 4, 4096, 128
N_CORES = 8
ROWS = N * H // N_CORES          # 2048 rows (n) per core
CHUNK = 1024                     # n columns processed per outer chunk
CHUNKS = ROWS // CHUNK           # 2
M_TILES = N // 128               # 32 tiles along the softmax (m) axis
SUBS = CHUNK // 128              # 8 PV subtiles per chunk
MB = 4                           # m-tiles per batched mask DMA
QKW = 512                        # QK matmul rhs width (one PSUM bank)

# Schraudolph constants (fp16 target: 10 mantissa bits)
SK = 1024.0 / float(np.log(2.0))     # 1477.32
BE = 15296.0                          # exp bias for E tiles (fp16-exact)
BC = 15299.0                          # exp bias for C tiles
E_MASKED = -29024.0                   # fp16(BE - 30*SK)
C_MASKED = -44320.0                   # fp16(-30*SK)

# Module-level knobs used by test.py; harmless defaults for grading.
TRACE = False
LAST_EXEC_NS = None

_CACHED_NC = None
_CACHED_BIAS = None


def _classes(has_bias, chunk):
    """Per-m-tile pipeline class: 'A' exact, 'E'/'C' Schraudolph."""
    if has_bias:
        return ["A"] * M_TILES
    cls = ["E"] * M_TILES
    if chunk == 0:
        # First tiles on ACT: compute starts before the mask DMAs land.
        # All classes are PAIR-aligned (EW ops cover two m-tiles at once);
        # chunks end on cheap-to-finish E pairs.
        a_set = (0, 1, 2, 3)
        c_set = (12, 13, 18, 19, 24, 25)
    else:
        # Masks are prefetched by now; lead with DVE-heavy tiles so DVE
        # ramps while ACT finishes the previous chunk.
        a_set = ()
        c_set = (0, 1, 8, 9, 18, 19)
    for m in a_set:
        cls[m] = "A"
    for m in c_set:
        cls[m] = "C"
    return cls


def _build_nc(has_bias=False):
    nc = bacc.Bacc("TRN2", target_bir_lowering=False, debug=False,
                   num_devices=N_CORES)
    xt_d = nc.dram_tensor("xt", [128, ROWS], BF, kind="ExternalInput").ap()
    wt_d = nc.dram_tensor("wt", [128, N], BF, kind="ExternalInput").ap()
    xa_d = nc.dram_tensor("xa", [N, D + 1], F16, kind="ExternalInput").ap()
    mk_d = nc.dram_tensor("mk", [N, ROWS], U16, kind="ExternalInput").ap()
    bc_d = nc.dram_tensor("bc", [128, M_TILES], F32, kind="ExternalInput").ap()
    out_d = nc.dram_tensor("out", [ROWS, D], F32, kind="ExternalOutput").ap()

    PRELU = mybir.ActivationFunctionType.Prelu
    EXP = mybir.ActivationFunctionType.Exp
    COPY = mybir.ActivationFunctionType.Copy
    MUL = mybir.AluOpType.mult
    ADD = mybir.AluOpType.add
    MAX = mybir.AluOpType.max

    WPC = 512                        # wt DMA piece columns (4 m-tiles)
    n_wt = N // WPC                  # 8 stationary pieces
    XG = 8                           # m-tiles per xa group
    n_xa = M_TILES // XG             # 4 PV-moving groups

    with tile.TileContext(nc) as tc:
        with (
            tc.tile_pool(name="const", bufs=1) as cpool,
            tc.tile_pool(name="mask", bufs=4) as mpool,
            tc.tile_pool(name="worka", bufs=3) as apool,
            tc.tile_pool(name="worke", bufs=3) as epool,
            tc.tile_pool(name="workeb", bufs=6) as ebpool,
            tc.tile_pool(name="workc", bufs=3) as qpool,
            tc.tile_pool(name="outp", bufs=3) as opool,
            tc.tile_pool(name="spsum", bufs=2, space="PSUM") as spool,
            tc.tile_pool(name="opsum", bufs=1, space="PSUM") as oppool,
        ):
            wt_sb = [cpool.tile([128, WPC], BF, name=f"wt{i}")
                     for i in range(n_wt)]
            xt_sb = [cpool.tile([128, CHUNK], BF, name=f"xt{i}")
                     for i in range(CHUNKS)]
            bc_sb = cpool.tile([128, M_TILES], F32)
            xa_sb = [cpool.tile([128, XG, D + 1], F16, name=f"xa{i}")
                     for i in range(n_xa)]

            # Warm the ACT LUT set while input DMAs are in flight.
            warm_sb = cpool.tile([1, 1], F32)
            nc.scalar.activation(warm_sb[:], warm_sb[:], EXP)

            # Only the pieces the first QK matmuls need go up front; the
            # rest are staggered inside the loop so early mask batches
            # are not starved of DMA bandwidth. The GpSimd sequencer comes
            # up ~2us before Sync, so the most urgent pieces issue there.
            nc.scalar.dma_start(out=wt_sb[0][:, 0:256], in_=wt_d[:, 0:256])
            nc.gpsimd.dma_start(out=xt_sb[0][:, 0:512], in_=xt_d[:, 0:512])
            nc.gpsimd.dma_start(out=wt_sb[0][:, 256:WPC],
                                in_=wt_d[:, 256:WPC])
            nc.sync.dma_start(out=xt_sb[0][:, 512:1024], in_=xt_d[:, 512:1024])
            if has_bias:
                nc.sync.dma_start(out=bc_sb[:], in_=bc_d[:, :])
            xa_v = xa_d.rearrange("(g j p) d -> g p j d", p=128, j=XG)

            pending_tail = None
            N_PAIRS = M_TILES // 2
            for c in range(CHUNKS):
                CLS = _classes(has_bias, c)

                o_ps = []

                def o_ap(s, o_ps=o_ps):
                    return o_ps[s // 3][:, (s % 3) * 129:(s % 3) * 129 + 129]

                state = {"first": True}

                def emit_pv(m, lhs_t, c=c, o_ps=o_ps, state=state):
                    nonlocal pending_tail
                    if state["first"]:
                        state["first"] = False
                        if pending_tail is not None:
                            pending_tail()
                            pending_tail = None
                        o_ps.extend(
                            oppool.tile([128, 512], F32, tag=f"oacc{b}",
                                        name=f"oacc{b}_c{c}")
                            for b in range((SUBS + 2) // 3))
                    for s in range(SUBS):
                        nc.tensor.matmul(
                            o_ap(s),
                            lhsT=lhs_t[:, s * 128:(s + 1) * 128],
                            rhs=xa_sb[m // XG][:, m % XG],
                            start=(m == 0 and s % 3 == 0),
                            stop=(m == M_TILES - 1),
                            skip_group_check=True,
                        )

                # Software pipeline: each pair's PV matmuls (and the A
                # pairs' mask multiply) are deferred one pair behind, so
                # the Tensor queue never sits directly behind a long EW
                # chain. One entry pops per iteration to avoid bursts.
                pv_q = []   # (ready_pair, emit_fn) in m order

                def pop_pv(limit, max_n=1):
                    n = 0
                    while pv_q and pv_q[0][0] <= limit and n < max_n:
                        pv_q.pop(0)[1]()
                        n += 1

                # Mask DMA batches: small leading batches on chunk 0 so
                # the first EW consumers aren't starved while the bulk
                # input DMAs stream.
                if c == 0:
                    batches = [(0, 2), (2, 2)] + [(s, 4) for s in
                                                  range(4, M_TILES, 4)]
                else:
                    batches = [(s, 4) for s in range(0, M_TILES, 4)]
                batch_at = {s: (i, sz) for i, (s, sz) in enumerate(batches)}

                mk_prs = {}
                for p in range(N_PAIRS):
                    m = 2 * p
                    pop_pv(p - 5)
                    if m in batch_at:
                        bi, bsz = batch_at[m]
                        mkb_sb = mpool.tile([128, bsz, CHUNK], U16,
                                            tag=f"mkb{bsz}",
                                            name=f"mkb_c{c}_m{m}")
                        mk_v = mk_d[m * 128:(m + bsz) * 128,
                                    c * CHUNK:(c + 1) * CHUNK].rearrange(
                                        "(t p) n -> p t n", p=128)
                        eng = nc.gpsimd if bi % 2 == 0 else nc.sync
                        eng.dma_start(out=mkb_sb[:], in_=mk_v)
                        for pp in range(m // 2, (m + bsz) // 2):
                            mk_prs[pp] = mkb_sb[:, 2 * pp - m:2 * pp - m + 2]
                    if c == 0:
                        # Staggered prefetch of the remaining inputs.
                        if p % 2 == 1 and p < 15:
                            k = (p + 1) // 2
                            nc.sync.dma_start(
                                out=wt_sb[k][:],
                                in_=wt_d[:, k * WPC:(k + 1) * WPC])
                        if p == 0:
                            nc.gpsimd.dma_start(out=xa_sb[0][:], in_=xa_v[0])
                        if p == 4:
                            for g in range(1, n_xa):
                                nc.gpsimd.dma_start(out=xa_sb[g][:],
                                                    in_=xa_v[g])
                        if p in (8, 9):
                            h = p - 8
                            nc.sync.dma_start(
                                out=xt_sb[1][:, h * 512:(h + 1) * 512],
                                in_=xt_d[:, CHUNK + h * 512:
                                         CHUNK + (h + 1) * 512])
                    mk_sb = mk_prs[p]          # [128, 2, CHUNK]
                    cls = CLS[m]

                    # QK per tile into single [128, CHUNK] PSUM tiles
                    # (2-deep ring keeps QK(m+2) overlapped with EW(m));
                    # the PSUM-evacuating op runs per tile right after its
                    # QK, writing into a shared SBUF pair tile so every
                    # downstream op covers 2048 elements.
                    s_list = []
                    if cls == "A":
                        pair_sb = apool.tile([128, 2, CHUNK], F16, tag="lq",
                                             name=f"lq_c{c}_p{p}")
                    elif cls == "E":
                        pair_sb = epool.tile([128, 2, CHUNK], F16, tag="lp",
                                             name=f"lp_c{c}_p{p}")
                    else:
                        pair_sb = qpool.tile([128, 2, CHUNK], F16, tag="ct",
                                             name=f"t_c{c}_p{p}")
                    for t in range(2):
                        mt = m + t
                        s_ps = spool.tile([128, CHUNK], F32, tag="s",
                                          name=f"s_c{c}_m{mt}")
                        for half in range(CHUNK // QKW):
                            nc.tensor.matmul(
                                s_ps[:, half * QKW:(half + 1) * QKW],
                                lhsT=wt_sb[(mt * 128) // WPC]
                                     [:, (mt * 128) % WPC:
                                      (mt * 128) % WPC + 128],
                                rhs=xt_sb[c][:, half * QKW:(half + 1) * QKW],
                                start=True, stop=True,
                            )
                        if cls == "A":
                            bias = (bc_sb[:, mt:mt + 1] if has_bias else 0.0)
                            nc.scalar.activation(pair_sb[:, t], s_ps[:],
                                                 PRELU, bias=bias,
                                                 scale=1.0, alpha=0.2)
                        elif cls == "E":
                            nc.scalar.activation(pair_sb[:, t], s_ps[:],
                                                 PRELU, bias=0.0,
                                                 scale=SK, alpha=0.2)
                        else:
                            nc.vector.scalar_tensor_tensor(
                                pair_sb[:, t], s_ps[:], 0.2 * SK,
                                mk_sb[:, t].bitcast(F16),
                                op0=MUL, op1=ADD)

                    if cls == "A":
                        eq_sb = apool.tile([128, 2, CHUNK], F16, tag="eq",
                                           name=f"eq_c{c}_p{p}")
                        nc.scalar.activation(eq_sb[:], pair_sb[:], EXP)

                        def go_a(p=p, m=m, eq_sb=eq_sb, mk_sb=mk_sb):
                            a_sb = epool.tile([128, 2, CHUNK], F16, tag="a",
                                              name=f"a_c{c}_p{p}")
                            nc.vector.tensor_tensor(
                                a_sb[:], eq_sb[:], mk_sb.bitcast(F16), MUL)
                            for t in range(2):
                                emit_pv(m + t, a_sb[:, t])
                        pv_q.append((p, go_a))
                    elif cls == "E":
                        eb_sb = ebpool.tile([128, 2, CHUNK], U16, tag="eb",
                                            name=f"eb_c{c}_p{p}")
                        nc.vector.tensor_tensor(eb_sb[:], pair_sb[:],
                                                mk_sb.bitcast(F16), ADD)

                        def go_e(p=p, m=m, eb_sb=eb_sb):
                            for t in range(2):
                                emit_pv(m + t, eb_sb[:, t].bitcast(F16))
                        pv_q.append((p, go_e))
                    else:  # C
                        u_sb = qpool.tile([128, 2, CHUNK], F16, tag="cu",
                                          name=f"u_c{c}_p{p}")
                        nc.vector.scalar_tensor_tensor(
                            u_sb[:], pair_sb[:], 5.0, pair_sb[:],
                            op0=MUL, op1=MAX)
                        cb_sb = qpool.tile([128, 2, CHUNK], U16, tag="cb",
                                           name=f"cb_c{c}_p{p}")
                        nc.vector.tensor_scalar(cb_sb[:], u_sb[:], BC, 0.0,
                                                op0=ADD, op1=MAX)

                        def go_c(p=p, m=m, cb_sb=cb_sb):
                            for t in range(2):
                                emit_pv(m + t, cb_sb[:, t].bitcast(F16))
                        pv_q.append((p, go_c))
                pop_pv(N_PAIRS, max_n=len(pv_q))

                def make_tail(c=c, o_ap=o_ap):
                    def emit_tail():
                        # Division tail: reciprocal of the free row-sum,
                        # scale-copy gathered into one tile per chunk.
                        of_big = opool.tile([128, SUBS, D], F32, tag="ofbig",
                                            name=f"ofbig_c{c}")
                        r_sbs = []
                        for s in range(SUBS):
                            r_sb = opool.tile([128, 1], F32, tag=f"recip{s}",
                                              name=f"recip_c{c}_s{s}")
                            nc.vector.reciprocal(r_sb[:], o_ap(s)[:, 128:129])
                            r_sbs.append(r_sb)
                        for s in range(SUBS):
                            on_act_scale = (c == CHUNKS - 1) and s % 2 == 0
                            if on_act_scale:
                                nc.scalar.activation(of_big[:, s],
                                                     o_ap(s)[:, 0:D],
                                                     COPY, bias=0.0,
                                                     scale=r_sbs[s][:])
                            else:
                                nc.vector.tensor_scalar_mul(of_big[:, s],
                                                            o_ap(s)[:, 0:D],
                                                            r_sbs[s][:])
                        pieces = 4 if c == CHUNKS - 1 else 1
                        hs = SUBS // pieces
                        engs = [nc.sync, nc.scalar, nc.gpsimd, nc.sync]
                        for hh in range(pieces):
                            row0 = c * CHUNK + hh * hs * 128
                            out_v = out_d[row0:row0 + hs * 128, :].rearrange(
                                "(s p) d -> p s d", p=128)
                            engs[hh].dma_start(
                                out=out_v,
                                in_=of_big[:, hh * hs:(hh + 1) * hs])
                    return emit_tail
                pending_tail = make_tail()
            if pending_tail is not None:
                pending_tail()

    nc.compile()
    return nc


def _pack_mask(mask_t_u8, has_bias):
    """mask_t_u8: [N, ROWS] 0/1. Returns uint16-packed per-m-tile rows."""
    out = np.empty(mask_t_u8.shape, np.uint16)
    for c in range(CHUNKS):
        cls = _classes(has_bias, c)
        cols = slice(c * CHUNK, (c + 1) * CHUNK)
        for m in range(M_TILES):
            rows = slice(m * 128, (m + 1) * 128)
            blk = mask_t_u8[rows, cols].astype(np.float32)
            if cls[m] == "A":
                vals = blk
            elif cls[m] == "E":
                vals = np.where(blk > 0, BE, E_MASKED)
            else:
                vals = np.where(blk > 0, 0.0, C_MASKED)
            out[rows, cols] = vals.astype(np.float16).view(np.uint16)
    return out


def kernel(x, W, b, neighbor_mask):
    global _CACHED_NC, _CACHED_BIAS, LAST_EXEC_NS
    x = np.asarray(x, dtype=np.float32)
    W = np.asarray(W, dtype=np.float32)
    b = np.asarray(b, dtype=np.float32)
    mask = np.asarray(neighbor_mask)

    has_bias = bool(np.any(b))
    if _CACHED_NC is None or _CACHED_BIAS != has_bias:
        _CACHED_NC = _build_nc(has_bias=has_bias)
        _CACHED_BIAS = has_bias
    nc = _CACHED_NC

    mask_u8 = mask.astype(np.uint8)
    in_maps = []
    for core in range(N_CORES):
        h, rb = core // 2, core % 2
        r0 = rb * ROWS
        xt = np.ascontiguousarray(x[h, r0:r0 + ROWS].T).astype(BF_NP)
        wt = np.ascontiguousarray(W[h].T).astype(BF_NP)
        xa = np.concatenate(
            [x[h], np.ones((N, 1), np.float32)], axis=1
        ).astype(np.float16)
        mk = _pack_mask(
            np.ascontiguousarray(mask_u8[r0:r0 + ROWS].T), has_bias)
        bc = np.ascontiguousarray(b[h].reshape(M_TILES, 128).T)
        in_maps.append({"xt": xt, "wt": wt, "xa": xa, "mk": mk, "bc": bc})

    res = run_bass_kernel_spmd(nc, in_maps, core_ids=list(range(N_CORES)),
                               trace=TRACE)
    LAST_EXEC_NS = res.exec_time_ns

    out = np.empty((H, N, D), np.float32)
    for core in range(N_CORES):
        h, rb = core // 2, core % 2
        r0 = rb * ROWS
        out[h, r0:r0 + ROWS] = res.results[core]["out"]
    return out


# revision 73
# speedup vs baseline: 1.0230x; 1.0230x over previous
"""Distributed Trainium2 (Bass/Tile) kernel for masked GAT-style attention.

Reference computation (H=4 heads, N=4096 nodes, D=128):
    scores = leaky_relu(x @ W^T + b, 0.2)            # [H, N, N]
    att    = where(mask, softmax(where(mask, scores, -inf)), 0)
    out    = att @ x                                  # [H, N, D]

Sharding: 8 cores = 4 heads x 2 row-blocks of 2048 nodes. Each core
computes out[h, r0:r0+2048] independently (no collectives).

Per-core layout ("transposed scores"): scores^T tiles [m=128 part, n free]
so the PV matmul uses the attention tile directly as the stationary
operand and the softmax row-sum comes for free from an appended
ones-column on x.

The 32 m-tiles per chunk are processed as 16 PAIRS (every SBUF-side EW
op covers 2048 elements, halving per-instruction overhead), split into
three pipelines chosen to balance ACT (1.2 GHz, 1x) and DVE (0.96 GHz,
STT=1x, TT=2x @16-bit):
- A (3 pairs): ACT Prelu from PSUM per tile -> one shared pair Exp on
  ACT -> fp16 mask multiply on DVE (2x TT). Exact exp.
- E (10 pairs): ACT Prelu(scale=K) gives K*leaky(s) in fp16; one DVE
  pair TT-add with a host-baked fp16 mask tensor {B, B-30K} produces
  Schraudolph exp bits in uint16 (negative results saturate to 0 =
  masked-out entries vanish); bitcast fp16 feeds the PV directly.
- C (3 pairs): all-DVE. STT t=0.2K*s+mask per tile (PSUM read), then a
  pair STT u=max(5t,t) and a 4x-mode tensor_scalar add-B -> uint16
  Schraudolph bits.
Schraudolph tiles (E+C) trade ~2-3% per-entry exp noise (which largely
cancels in the softmax) for removing the ACT exp pass; measured
end-to-end rel err 1.39e-2 vs the 2e-2 gate.

Each pair's PV matmuls (and the A pairs' mask multiply) are deferred
five pairs behind the QK/EW front (a software pipeline queue, one pop
per iteration), so the Tensor queue never sits directly behind a long
EW chain and jitter doesn't couple through the 2-deep s_ps PSUM ring.

Mask ships once as uint16 [N, ROWS]: fp16 1.0/0.0 rows for A tiles,
fp16 {B, B-30K} for E, fp16 {0, -44320} for C; bitcast on chip.
Mask/xa DMAs are batched and issued from the otherwise-idle GpSimd
sequencer (alternating with Sync); wt/xt stream in 512-col pieces
staggered through chunk 0 so early mask batches aren't starved and the
first QK matmul starts as soon as the engines come up.
"""

import sys

if "/opt/trn_rl_repo" not in sys.path:
    sys.path.insert(0, "/opt/trn_rl_repo")

import numpy as np
import ml_dtypes

import concourse.bass as bass
import concourse.tile as tile
from concourse import bacc, mybir
from concourse.bass_utils import run_bass_kernel_spmd

BF = mybir.dt.bfloat16
F16 = mybir.dt.float16
F32 = mybir.dt.float32
U16 = mybir.dt.uint16
BF_NP = ml_dtypes.bfloat16

H, N, D = 4, 4096, 128
N_CORES = 8
ROWS = N * H // N_CORES          # 2048 rows (n) per core
CHUNK = 1024                     # n columns processed per outer chunk
CHUNKS = ROWS // CHUNK           # 2
M_TILES = N // 128               # 32 tiles along the softmax (m) axis
SUBS = CHUNK // 128              # 8 PV subtiles per chunk
MB = 4                           # m-tiles per batched mask DMA
QKW = 512                        # QK matmul rhs width (one PSUM bank)

# Schraudolph constants (fp16 target: 10 mantissa bits)
SK = 1024.0 / float(np.log(2.0))     # 1477.32
BE = 15296.0                          # exp bias for E tiles (fp16-exact)
BC = 15299.0                          # exp bias for C tiles
E_MASKED = -29024.0                   # fp16(BE - 30*SK)
C_MASKED = -44320.0                   # fp16(-30*SK)

# Module-level knobs used by test.py; harmless defaults for grading.
TRACE = False
LAST_EXEC_NS = None

_CACHED_NC = None
_CACHED_BIAS = None


def _classes(has_bias, chunk):
    """Per-m-tile pipeline class: 'A' exact, 'E'/'C' Schraudolph."""
    if has_bias:
        return ["A"] * M_TILES
    cls = ["E"] * M_TILES
    if chunk == 0:
        # First tiles on ACT: compute starts before the mask DMAs land.
        # All classes are PAIR-aligned (EW ops cover two m-tiles at once);
        # chunks end on cheap-to-finish E pairs.
        a_set = (0, 1, 2, 3)
        c_set = (12, 13, 18, 19, 24, 25)
    else:
        # Masks are prefetched by now; lead with DVE-heavy tiles so DVE
        # ramps while ACT finishes the previous chunk.
        a_set = ()
        c_set = (0, 1, 8, 9, 18, 19)
    for m in a_set:
        cls[m] = "A"
    for m in c_set:
        cls[m] = "C"
    return cls


def _build_nc(has_bias=False):
    nc = bacc.Bacc("TRN2", target_bir_lowering=False, debug=False,
                   num_devices=N_CORES)
    xt_d = nc.dram_tensor("xt", [128, ROWS], BF, kind="ExternalInput").ap()
    wt_d = nc.dram_tensor("wt", [128, N], BF, kind="ExternalInput").ap()
    xa_d = nc.dram_tensor("xa", [N, D + 1], F16, kind="ExternalInput").ap()
    mk_d = nc.dram_tensor("mk", [N, ROWS], U16, kind="ExternalInput").ap()
    bc_d = nc.dram_tensor("bc", [128, M_TILES], F32, kind="ExternalInput").ap()
    out_d = nc.dram_tensor("out", [ROWS, D], F32, kind="ExternalOutput").ap()

    PRELU = mybir.ActivationFunctionType.Prelu
    EXP = mybir.ActivationFunctionType.Exp
    COPY = mybir.ActivationFunctionType.Copy
    MUL = mybir.AluOpType.mult
    ADD = mybir.AluOpType.add
    MAX = mybir.AluOpType.max

    WPC = 512                        # wt DMA piece columns (4 m-tiles)
    n_wt = N // WPC                  # 8 stationary pieces
    XG = 8                           # m-tiles per xa group
    n_xa = M_TILES // XG             # 4 PV-moving groups

    with tile.TileContext(nc) as tc:
        with (
            tc.tile_pool(name="const", bufs=1) as cpool,
            tc.tile_pool(name="mask", bufs=4) as mpool,
            tc.tile_pool(name="worka", bufs=3) as apool,
            tc.tile_pool(name="worke", bufs=3) as epool,
            tc.tile_pool(name="workeb", bufs=6) as ebpool,
            tc.tile_pool(name="workc", bufs=3) as qpool,
            tc.tile_pool(name="outp", bufs=3) as opool,
            tc.tile_pool(name="spsum", bufs=2, space="PSUM") as spool,
            tc.tile_pool(name="opsum", bufs=1, space="PSUM") as oppool,
        ):
            wt_sb = [cpool.tile([128, WPC], BF, name=f"wt{i}")
                     for i in range(n_wt)]
            xt_sb = [cpool.tile([128, CHUNK], BF, name=f"xt{i}")
                     for i in range(CHUNKS)]
            bc_sb = cpool.tile([128, M_TILES], F32)
            xa_sb = [cpool.tile([128, XG, D + 1], F16, name=f"xa{i}")
                     for i in range(n_xa)]

            # Warm the ACT LUT set while input DMAs are in flight.
            warm_sb = cpool.tile([1, 1], F32)
            nc.scalar.activation(warm_sb[:], warm_sb[:], EXP)

            # Only the pieces the first QK matmuls need go up front; the
            # rest are staggered inside the loop so early mask batches
            # are not starved of DMA bandwidth. The GpSimd sequencer comes
            # up ~2us before Sync, so the most urgent pieces issue there.
            nc.gpsimd.dma_start(out=wt_sb[0][:], in_=wt_d[:, 0:WPC])
            nc.gpsimd.dma_start(out=xt_sb[0][:, 0:512], in_=xt_d[:, 0:512])
            nc.sync.dma_start(out=xt_sb[0][:, 512:1024], in_=xt_d[:, 512:1024])
            if has_bias:
                nc.sync.dma_start(out=bc_sb[:], in_=bc_d[:, :])
            xa_v = xa_d.rearrange("(g j p) d -> g p j d", p=128, j=XG)

            pending_tail = None
            N_PAIRS = M_TILES // 2
            for c in range(CHUNKS):
                CLS = _classes(has_bias, c)

                o_ps = []

                def o_ap(s, o_ps=o_ps):
                    return o_ps[s // 3][:, (s % 3) * 129:(s % 3) * 129 + 129]

                state = {"first": True}

                def emit_pv(m, lhs_t, c=c, o_ps=o_ps, state=state):
                    nonlocal pending_tail
                    if state["first"]:
                        state["first"] = False
                        if pending_tail is not None:
                            pending_tail()
                            pending_tail = None
                        o_ps.extend(
                            oppool.tile([128, 512], F32, tag=f"oacc{b}",
                                        name=f"oacc{b}_c{c}")
                            for b in range((SUBS + 2) // 3))
                    for s in range(SUBS):
                        nc.tensor.matmul(
                            o_ap(s),
                            lhsT=lhs_t[:, s * 128:(s + 1) * 128],
                            rhs=xa_sb[m // XG][:, m % XG],
                            start=(m == 0 and s % 3 == 0),
                            stop=(m == M_TILES - 1),
                            skip_group_check=True,
                        )

                # Software pipeline: each pair's PV matmuls (and the A
                # pairs' mask multiply) are deferred one pair behind, so
                # the Tensor queue never sits directly behind a long EW
                # chain. One entry pops per iteration to avoid bursts.
                pv_q = []   # (ready_pair, emit_fn) in m order

                def pop_pv(limit, max_n=1):
                    n = 0
                    while pv_q and pv_q[0][0] <= limit and n < max_n:
                        pv_q.pop(0)[1]()
                        n += 1

                # Mask DMA batches: small leading batches on chunk 0 so
                # the first EW consumers aren't starved while the bulk
                # input DMAs stream.
                if c == 0:
                    batches = [(0, 2), (2, 2)] + [(s, 4) for s in
                                                  range(4, M_TILES, 4)]
                else:
                    batches = [(s, 4) for s in range(0, M_TILES, 4)]
                batch_at = {s: (i, sz) for i, (s, sz) in enumerate(batches)}

                mk_prs = {}
                for p in range(N_PAIRS):
                    m = 2 * p
                    pop_pv(p - 5)
                    if m in batch_at:
                        bi, bsz = batch_at[m]
                        mkb_sb = mpool.tile([128, bsz, CHUNK], U16,
                                            tag=f"mkb{bsz}",
                                            name=f"mkb_c{c}_m{m}")
                        mk_v = mk_d[m * 128:(m + bsz) * 128,
                                    c * CHUNK:(c + 1) * CHUNK].rearrange(
                                        "(t p) n -> p t n", p=128)
                        eng = nc.gpsimd if bi % 2 == 0 else nc.sync
                        eng.dma_start(out=mkb_sb[:], in_=mk_v)
                        for pp in range(m // 2, (m + bsz) // 2):
                            mk_prs[pp] = mkb_sb[:, 2 * pp - m:2 * pp - m + 2]
                    if c == 0:
                        # Staggered prefetch of the remaining inputs.
                        if p % 2 == 1 and p < 15:
                            k = (p + 1) // 2
                            nc.sync.dma_start(
                                out=wt_sb[k][:],
                                in_=wt_d[:, k * WPC:(k + 1) * WPC])
                        if p == 0:
                            nc.gpsimd.dma_start(out=xa_sb[0][:], in_=xa_v[0])
                        if p == 4:
                            for g in range(1, n_xa):
                                nc.gpsimd.dma_start(out=xa_sb[g][:],
                                                    in_=xa_v[g])
                        if p in (8, 9):
                            h = p - 8
                            nc.sync.dma_start(
                                out=xt_sb[1][:, h * 512:(h + 1) * 512],
                                in_=xt_d[:, CHUNK + h * 512:
                                         CHUNK + (h + 1) * 512])
                    mk_sb = mk_prs[p]          # [128, 2, CHUNK]
                    cls = CLS[m]

                    # QK per tile into single [128, CHUNK] PSUM tiles
                    # (2-deep ring keeps QK(m+2) overlapped with EW(m));
                    # the PSUM-evacuating op runs per tile right after its
                    # QK, writing into a shared SBUF pair tile so every
                    # downstream op covers 2048 elements.
                    s_list = []
                    if cls == "A":
                        pair_sb = apool.tile([128, 2, CHUNK], F16, tag="lq",
                                             name=f"lq_c{c}_p{p}")
                    elif cls == "E":
                        pair_sb = epool.tile([128, 2, CHUNK], F16, tag="lp",
                                             name=f"lp_c{c}_p{p}")
                    else:
                        pair_sb = qpool.tile([128, 2, CHUNK], F16, tag="ct",
                                             name=f"t_c{c}_p{p}")
                    for t in range(2):
                        mt = m + t
                        s_ps = spool.tile([128, CHUNK], F32, tag="s",
                                          name=f"s_c{c}_m{mt}")
                        for half in range(CHUNK // QKW):
                            nc.tensor.matmul(
                                s_ps[:, half * QKW:(half + 1) * QKW],
                                lhsT=wt_sb[(mt * 128) // WPC]
                                     [:, (mt * 128) % WPC:
                                      (mt * 128) % WPC + 128],
                                rhs=xt_sb[c][:, half * QKW:(half + 1) * QKW],
                                start=True, stop=True,
                            )
                        if cls == "A":
                            bias = (bc_sb[:, mt:mt + 1] if has_bias else 0.0)
                            nc.scalar.activation(pair_sb[:, t], s_ps[:],
                                                 PRELU, bias=bias,
                                                 scale=1.0, alpha=0.2)
                        elif cls == "E":
                            nc.scalar.activation(pair_sb[:, t], s_ps[:],
                                                 PRELU, bias=0.0,
                                                 scale=SK, alpha=0.2)
                        else:
                            nc.vector.scalar_tensor_tensor(
                                pair_sb[:, t], s_ps[:], 0.2 * SK,
                                mk_sb[:, t].bitcast(F16),
                                op0=MUL, op1=ADD)

                    if cls == "A":
                        eq_sb = apool.tile([128, 2, CHUNK], F16, tag="eq",
                                           name=f"eq_c{c}_p{p}")
                        nc.scalar.activation(eq_sb[:], pair_sb[:], EXP)

                        def go_a(p=p, m=m, eq_sb=eq_sb, mk_sb=mk_sb):
                            a_sb = epool.tile([128, 2, CHUNK], F16, tag="a",
                                              name=f"a_c{c}_p{p}")
                            nc.vector.tensor_tensor(
                                a_sb[:], eq_sb[:], mk_sb.bitcast(F16), MUL)
                            for t in range(2):
                                emit_pv(m + t, a_sb[:, t])
                        pv_q.append((p, go_a))
                    elif cls == "E":
                        eb_sb = ebpool.tile([128, 2, CHUNK], U16, tag="eb",
                                            name=f"eb_c{c}_p{p}")
                        nc.vector.tensor_tensor(eb_sb[:], pair_sb[:],
                                                mk_sb.bitcast(F16), ADD)

                        def go_e(p=p, m=m, eb_sb=eb_sb):
                            for t in range(2):
                                emit_pv(m + t, eb_sb[:, t].bitcast(F16))
                        pv_q.append((p, go_e))
                    else:  # C
                        u_sb = qpool.tile([128, 2, CHUNK], F16, tag="cu",
                                          name=f"u_c{c}_p{p}")
                        nc.vector.scalar_tensor_tensor(
                            u_sb[:], pair_sb[:], 5.0, pair_sb[:],
                            op0=MUL, op1=MAX)
                        cb_sb = qpool.tile([128, 2, CHUNK], U16, tag="cb",
                                           name=f"cb_c{c}_p{p}")
                        nc.vector.tensor_scalar(cb_sb[:], u_sb[:], BC, 0.0,
                                                op0=ADD, op1=MAX)

                        def go_c(p=p, m=m, cb_sb=cb_sb):
                            for t in range(2):
                                emit_pv(m + t, cb_sb[:, t].bitcast(F16))
                        pv_q.append((p, go_c))
                pop_pv(N_PAIRS, max_n=len(pv_q))

                def make_tail(c=c, o_ap=o_ap):
                    def emit_tail():
                        # Division tail: reciprocal of the free row-sum,
                        # scale-copy gathered into one tile per chunk.
                        of_big = opool.tile([128, SUBS, D], F32, tag="ofbig",
                                            name=f"ofbig_c{c}")
                        r_sbs = []
                        for s in range(SUBS):
                            r_sb = opool.tile([128, 1], F32, tag=f"recip{s}",
                                              name=f"recip_c{c}_s{s}")
                            nc.vector.reciprocal(r_sb[:], o_ap(s)[:, 128:129])
                            r_sbs.append(r_sb)
                        for s in range(SUBS):
                            on_act_scale = (c == CHUNKS - 1) and s % 2 == 0
                            if on_act_scale:
                                nc.scalar.activation(of_big[:, s],
                                                     o_ap(s)[:, 0:D],
                                                     COPY, bias=0.0,
                                                     scale=r_sbs[s][:])
                            else:
                                nc.vector.tensor_scalar_mul(of_big[:, s],
                                                            o_ap(s)[:, 0:D],
                                                            r_sbs[s][:])
                        pieces = 4 if c == CHUNKS - 1 else 1
                        hs = SUBS // pieces
                        engs = [nc.sync, nc.scalar, nc.gpsimd, nc.sync]
                        for hh in range(pieces):
                            row0 = c * CHUNK + hh * hs * 128
                            out_v = out_d[row0:row0 + hs * 128, :].rearrange(
                                "(s p) d -> p s d", p=128)
                            engs[hh].dma_start(
                                out=out_v,
                                in_=of_big[:, hh * hs:(hh + 1) * hs])
                    return emit_tail
                pending_tail = make_tail()
            if pending_tail is not None:
                pending_tail()

    nc.compile()
    return nc


def _pack_mask(mask_t_u8, has_bias):
    """mask_t_u8: [N, ROWS] 0/1. Returns uint16-packed per-m-tile rows."""
    out = np.empty(mask_t_u8.shape, np.uint16)
    for c in range(CHUNKS):
        cls = _classes(has_bias, c)
        cols = slice(c * CHUNK, (c + 1) * CHUNK)
        for m in range(M_TILES):
            rows = slice(m * 128, (m + 1) * 128)
            blk = mask_t_u8[rows, cols].astype(np.float32)
            if cls[m] == "A":
                vals = blk
            elif cls[m] == "E":
                vals = np.where(blk > 0, BE, E_MASKED)
            else:
                vals = np.where(blk > 0, 0.0, C_MASKED)
            out[rows, cols] = vals.astype(np.float16).view(np.uint16)
    return out


def kernel(x, W, b, neighbor_mask):
    global _CACHED_NC, _CACHED_BIAS, LAST_EXEC_NS
    x = np.asarray(x, dtype=np.float32)
    W = np.asarray(W, dtype=np.float32)
    b = np.asarray(b, dtype=np.float32)
    mask = np.asarray(neighbor_mask)

    has_bias = bool(np.any(b))
    if _CACHED_NC is None or _CACHED_BIAS != has_bias:
        _CACHED_NC = _build_nc(has_bias=has_bias)
        _CACHED_BIAS = has_bias
    nc = _CACHED_NC

    mask_u8 = mask.astype(np.uint8)
    in_maps = []
    for core in range(N_CORES):
        h, rb = core // 2, core % 2
        r0 = rb * ROWS
        xt = np.ascontiguousarray(x[h, r0:r0 + ROWS].T).astype(BF_NP)
        wt = np.ascontiguousarray(W[h].T).astype(BF_NP)
        xa = np.concatenate(
            [x[h], np.ones((N, 1), np.float32)], axis=1
        ).astype(np.float16)
        mk = _pack_mask(
            np.ascontiguousarray(mask_u8[r0:r0 + ROWS].T), has_bias)
        bc = np.ascontiguousarray(b[h].reshape(M_TILES, 128).T)
        in_maps.append({"xt": xt, "wt": wt, "xa": xa, "mk": mk, "bc": bc})

    res = run_bass_kernel_spmd(nc, in_maps, core_ids=list(range(N_CORES)),
                               trace=TRACE)
    LAST_EXEC_NS = res.exec_time_ns

    out = np.empty((H, N, D), np.float32)
    for core in range(N_CORES):
        h, rb = core // 2, core % 2
        r0 = rb * ROWS
        out[h, r0:r0 + ROWS] = res.results[core]["out"]
    return out
